# revision 14
# baseline (speedup 1.0000x reference)
"""MoE routing kernel for Trainium2 (8 NeuronCores, I-sharded, fp8 comp).

Problem: nn_MoDE_52140902973544 (moe_routing).
  x[4,2048,1024], router (8 experts, top-2, capacity 1024), 7 real experts
  with FFN H=1024 -> I=4096 -> H=1024 (relu), expert 7 = identity (noop).

Strategy:
  * Host: router forward + top-2 + capacity-limited dispatch (pure index
    math, order-based -> float-robust).  Within each expert the CAP
    dispatched slots are PERMUTED by importance (gate weight x token
    norm, descending): the combine is a gather so any permutation is
    exact; it lets the device compute the low-importance tail cheaper.
  * Device (SPMD over 8 cores): core c owns I-slice [c*512,(c+1)*512)
    of the FFN intermediate dim for ALL 7 experts (perfect balance, no
    collectives); GEMM2 emits partial sums over I, summed on host.
    All matmuls are fp8e4m3 in DoubleRow perf mode (4x bf16 column
    rate: half the cycles per column x half the instructions).
  * Precision: segment A (top-importance NB tokens/expert) uses 3-term
    error compensation -- x = xh + xl (fp8 pair), W = Wh + Wl where Wl
    is the unscaled fp8 residual (subnormal range), computing
        Wh.T@xh + Wh.T@xl + Wl.T@xh        (lo*lo term dropped)
    in ONE PSUM chain: 0.75x bf16 PE cost at ~bf16 accuracy (the same
    3-term trick is applied in GEMM2 with h = hh + hl split on device).
    Segment B (NQ low-importance tokens) is raw fp8 (0.25x cost); its
    ~4e-2 relative noise lands on slots whose combined-output share is
    ~15%, keeping end-to-end error ~1.7e-2 (< 2e-2 gate; validated
    bit-accurately by the numpy emulation in err_lab.py).
  * B shares Wh/Woh tiles with A -- per-expert DMA is 3.5MB in, 2MB out
    (38.5MB/core/iter vs a measured ~360GB/s/core ceiling).
  * Host: sum the 8 bf16 partials in fp32, /1024 (fp8 scale), combine
    via pure gathers + gate weights + noop path.

fp8 scales (powers of 2, exponent-only): Wh = q8(16*Wi), Woh = q8(64*Wo)
lift the weights out of the fp8e4 subnormal range; the residuals Wl/Wol
live in the subnormal range deliberately (their quantization error is
second-order).  h16 = 16*h; outputs carry 16*64 = 1024x, undone on host.
"""

import os
import sys

for _p in ("/opt/trn_rl_repo", "/opt/pypackages"):
    if _p not in sys.path:
        sys.path.append(_p)

import numpy as np

# ---- problem constants (hardcoded per contract) ----
B, S, H, I = 4, 2048, 1024, 4096
E = 8                 # experts incl. noop (last)
ER = E - 1            # real experts
TOP_K = 2
N_TOK = B * S         # 8192
CAP = 1024            # ceil(N_TOK / E * 1.0)
N_CORES = 8

P = 128               # partitions
KO = H // P           # 8   H chunks
ISL = I // N_CORES    # 512 I-slice per core
ICH = ISL // P        # 4   I chunks per core
NF = 512              # max matmul free dim per PSUM tile (1 bank fp32)

# tokens per expert computed in raw fp8 (low-importance tail);
# the remaining NB = CAP - NQ use 3-term compensated fp8
NQ = int(os.environ.get("MOE_NQ", "512"))
NB = CAP - NQ
WI_SCALE = 16.0
WO_SCALE = 64.0
OUT_SCALE = 1.0 / (WI_SCALE * WO_SCALE)

MM_DTYPE = os.environ.get("MOE_MM_DTYPE", "bf16")

_CACHE = {}


def _free_tiles(n):
    """Split a free dim of n columns into <=NF chunks: [(off, width)...]"""
    out, off = [], 0
    while off < n:
        w = min(NF, n - off)
        out.append((off, w))
        off += w
    return out


def _build_nc(mm_dtype: str = "bf16", repeat: int = 1,
              loop_repeat: int | None = None, staggered: bool = False):
    """Single-core Bass program (SPMD across 8 cores, I-sharded).

    DRAM inputs host-packed with the partition dim first so every DMA is
    a plain contiguous slice (all fp8e4m3 except the bf16 outputs):
      xh,xl [128, 7*8, NB]   hi/lo split of segment-A tokens
      xb    [128, 7*8, NQ]   segment-B tokens
      wh,wl [128, 7*8, 512]  hi/lo of 16*Wi I-slice
      woh,wol [128, 7*4, 1024]  hi/lo of 64*Wo I-slice
      ypa [128, 7*8, NB] bf16 out; ypb [128, 7*8, NQ] bf16 out
    """
    import concourse.bacc as bacc
    import concourse.mybir as mybir
    import concourse.tile as tile

    dt = mybir.dt
    F8 = dt.float8e4
    DR = mybir.MatmulPerfMode.DoubleRow
    KS = 2

    nc = bacc.Bacc("TRN2")
    dp = nc.declare_dram_parameter
    if NB:
        xh = dp("xh", [P, ER * KO, NB], F8, isOutput=False)
        xl = dp("xl", [P, ER * KO, NB], F8, isOutput=False)
        wl = dp("wl", [P, ER * KO, ISL], F8, isOutput=False)
        wol = dp("wol", [P, ER * ICH, H], F8, isOutput=False)
        ypa = dp("ypa", [P, ER * KO, NB], dt.bfloat16, isOutput=True)
    if NQ:
        xb = dp("xb", [P, ER * KO, NQ], F8, isOutput=False)
        ypb = dp("ypb", [P, ER * KO, NQ], dt.bfloat16, isOutput=True)
    wh = dp("wh", [P, ER * KO, ISL], F8, isOutput=False)
    woh = dp("woh", [P, ER * ICH, H], F8, isOutput=False)

    ftsA = _free_tiles(NB)
    ftsB = _free_tiles(NQ)

    with tile.TileContext(nc) as tc:
        from contextlib import ExitStack

        with ExitStack() as ctx:
            xpool = ctx.enter_context(tc.tile_pool(name="x", bufs=2))
            wpool = ctx.enter_context(tc.tile_pool(name="w", bufs=2))
            hpool = ctx.enter_context(tc.tile_pool(name="h", bufs=2))
            spool = ctx.enter_context(tc.tile_pool(name="s", bufs=3))
            opool = ctx.enter_context(tc.tile_pool(name="o", bufs=3))
            # 8 one-bank [P, <=512] fp32 tiles: drains of chain g overlap
            # matmuls of later chains
            pspool = ctx.enter_context(
                tc.tile_pool(name="ps", bufs=8, space="PSUM"))

            def _dma_in(pool, shape, tag, name, src):
                t = pool.tile(shape, F8, tag=tag, name=name)
                nc.sync.dma_start(t[:], src)
                return t

            def _emit_expert(e):
                rk = slice(e * KO, (e + 1) * KO)
                rc = slice(e * ICH, (e + 1) * ICH)
                if NB:
                    xht = _dma_in(xpool, [P, KO, NB], "xh", f"xh{e}", xh[:, rk, :])
                    xlt = _dma_in(xpool, [P, KO, NB], "xl", f"xl{e}", xl[:, rk, :])
                    wlt = _dma_in(wpool, [P, KO, ISL], "wl", f"wl{e}", wl[:, rk, :])
                    wolt = _dma_in(wpool, [P, ICH, H], "wol", f"wol{e}", wol[:, rc, :])
                    hht = hpool.tile([P, ICH, NB], F8, tag="hh", name=f"hh{e}")
                    hlt = hpool.tile([P, ICH, NB], F8, tag="hl", name=f"hl{e}")
                if NQ:
                    xbt = _dma_in(xpool, [P, KO, NQ], "xb", f"xb{e}", xb[:, rk, :])
                    hbt = hpool.tile([P, ICH, NQ], F8, tag="hb", name=f"hb{e}")
                wht = _dma_in(wpool, [P, KO, ISL], "wh", f"wh{e}", wh[:, rk, :])
                woht = _dma_in(wpool, [P, ICH, H], "woh", f"woh{e}", woh[:, rc, :])

                # ---- GEMM1: h16 = relu(Wh.T@xh + Wh.T@xl + Wl.T@xh) ----
                for ir in range(0, ICH, 2):
                    if NB:
                        ps = [[pspool.tile([P, w], dt.float32, tag="ps",
                                           name=f"g1a_{e}_{ir + di}_{oi}")
                               for oi, (off, w) in enumerate(ftsA)]
                              for di in range(2)]
                        for pi, (lhs, rhs) in enumerate(
                                ((wht, xht), (wht, xlt), (wlt, xht))):
                            for k in range(0, KO, KS):
                                for di in range(2):
                                    for oi, (off, w) in enumerate(ftsA):
                                        nc.tensor.matmul(
                                            ps[di][oi][:],
                                            lhs[:, k:k + KS,
                                                (ir + di) * P:(ir + di + 1) * P],
                                            rhs[:, k:k + KS, off:off + w],
                                            start=(pi == 0 and k == 0),
                                            stop=(pi == 2 and k == KO - KS),
                                            perf_mode=DR,
                                        )
                        for di in range(2):
                            for oi, (off, w) in enumerate(ftsA):
                                st = spool.tile([P, w], dt.float32, tag="st",
                                                name=f"st_{e}_{ir + di}_{oi}")
                                nc.vector.tensor_scalar_max(
                                    st[:], ps[di][oi][:], 0.0)
                                nc.any.tensor_copy(
                                    hht[:, ir + di, off:off + w], st[:])
                                nc.vector.tensor_sub(
                                    hlt[:, ir + di, off:off + w], st[:],
                                    hht[:, ir + di, off:off + w])
                    if NQ:
                        psb = [[pspool.tile([P, w], dt.float32, tag="ps",
                                            name=f"g1b_{e}_{ir + di}_{oi}")
                                for oi, (off, w) in enumerate(ftsB)]
                               for di in range(2)]
                        for k in range(0, KO, KS):
                            for di in range(2):
                                for oi, (off, w) in enumerate(ftsB):
                                    nc.tensor.matmul(
                                        psb[di][oi][:],
                                        wht[:, k:k + KS,
                                            (ir + di) * P:(ir + di + 1) * P],
                                        xbt[:, k:k + KS, off:off + w],
                                        start=(k == 0),
                                        stop=(k == KO - KS),
                                        perf_mode=DR,
                                    )
                        for di in range(2):
                            for oi, (off, w) in enumerate(ftsB):
                                nc.vector.tensor_scalar_max(
                                    hbt[:, ir + di, off:off + w],
                                    psb[di][oi][:], 0.0)

                # ---- GEMM2: y = Woh.T@hh + Woh.T@hl + Wol.T@hh ----
                for hr in range(0, KO, 2):
                    if NB:
                        qs = [[pspool.tile([P, w], dt.float32, tag="ps",
                                           name=f"g2a_{e}_{hr + m}_{oi}")
                               for oi, (off, w) in enumerate(ftsA)]
                              for m in range(2)]
                        for pi, (lhs, rhs) in enumerate(
                                ((woht, hht), (woht, hlt), (wolt, hht))):
                            for k in range(0, ICH, KS):
                                for m in range(2):
                                    for oi, (off, w) in enumerate(ftsA):
                                        nc.tensor.matmul(
                                            qs[m][oi][:],
                                            lhs[:, k:k + KS,
                                                (hr + m) * P:(hr + m + 1) * P],
                                            rhs[:, k:k + KS, off:off + w],
                                            start=(pi == 0 and k == 0),
                                            stop=(pi == 2 and k == ICH - KS),
                                            perf_mode=DR,
                                        )
                        ot = opool.tile([P, 2, NB], dt.bfloat16, tag="oa",
                                        name=f"oa{e}_{hr}")
                        for m in range(2):
                            for oi, (off, w) in enumerate(ftsA):
                                nc.any.tensor_copy(
                                    ot[:, m, off:off + w], qs[m][oi][:])
                        nc.sync.dma_start(
                            ypa[:, e * KO + hr:e * KO + hr + 2, :], ot[:])
                    if NQ:
                        qsb = [[pspool.tile([P, w], dt.float32, tag="ps",
                                            name=f"g2b_{e}_{hr + m}_{oi}")
                                for oi, (off, w) in enumerate(ftsB)]
                               for m in range(2)]
                        for k in range(0, ICH, KS):
                            for m in range(2):
                                for oi, (off, w) in enumerate(ftsB):
                                    nc.tensor.matmul(
                                        qsb[m][oi][:],
                                        woht[:, k:k + KS,
                                             (hr + m) * P:(hr + m + 1) * P],
                                        hbt[:, k:k + KS, off:off + w],
                                        start=(k == 0),
                                        stop=(k == ICH - KS),
                                        perf_mode=DR,
                                    )
                        otb = opool.tile([P, 2, NQ], dt.bfloat16, tag="ob",
                                         name=f"ob{e}_{hr}")
                        for m in range(2):
                            for oi, (off, w) in enumerate(ftsB):
                                nc.any.tensor_copy(
                                    otb[:, m, off:off + w], qsb[m][oi][:])
                        nc.sync.dma_start(
                            ypb[:, e * KO + hr:e * KO + hr + 2, :], otb[:])

            def _emit_body():
                for e in range(ER):
                    _emit_expert(e)

            if loop_repeat is not None:
                # device-side repeat loop for the slope timing method
                with tc.For_i(0, loop_repeat, 1,
                              hint_engines=(mybir.EngineType.PE,
                                            mybir.EngineType.DVE),
                              staggered_reset=staggered):
                    _emit_body()
            else:
                for _rep in range(repeat):
                    _emit_body()
    nc.compile()
    return nc


def _get_nc(mm_dtype: str):
    if mm_dtype not in _CACHE:
        _CACHE[mm_dtype] = _build_nc(mm_dtype)
    return _CACHE[mm_dtype]


def _routing(x_flat: np.ndarray, router_w: np.ndarray, router_b: np.ndarray):
    """Replicate the reference router bit-for-bit where possible (jax CPU),
    returning top-2 values/indices [N_TOK, 2] (fp32/int)."""
    try:
        import jax
        import jax.numpy as jnp

        cpu = jax.devices("cpu")[0]
        with jax.default_device(cpu):
            xj = jnp.asarray(x_flat.reshape(B, S, H))
            logits = jnp.einsum("bsh,eh->bse", xj, jnp.asarray(router_w)) \
                + jnp.asarray(router_b)
            wflat = jax.nn.softmax(logits, axis=-1).reshape(N_TOK, E)
            topv, topi = jax.lax.top_k(wflat, TOP_K)
            return np.asarray(topv), np.asarray(topi)
    except Exception:
        # numpy fallback (float64 logits for a stable ordering)
        logits = x_flat.astype(np.float64) @ router_w.astype(np.float64).T \
            + router_b.astype(np.float64)
        m = logits.max(axis=1, keepdims=True)
        ex = np.exp(logits - m)
        wflat = (ex / ex.sum(axis=1, keepdims=True)).astype(np.float32)
        topi = np.argsort(-wflat, axis=1, kind="stable")[:, :TOP_K]
        topv = np.take_along_axis(wflat, topi, axis=1)
        return topv, topi


def _dispatch(x_flat, topv, topi):
    """Capacity-limited dispatch (exact reference order semantics), with
    slots permuted by importance inside each expert.

    Returns (pos, disp_T): pos[t, e] = slot column of token t for expert
    e (importance-permuted); disp_T[e] = x of the first CAP selectors in
    importance order, transposed to [H, CAP]."""
    mask = np.zeros((N_TOK, E), dtype=bool)
    rows = np.arange(N_TOK)
    mask[rows[:, None], topi] = True
    expert_mask = mask[:, :ER]                       # [N, 7]
    xnorm = np.linalg.norm(x_flat, axis=1)

    pos = np.full((N_TOK, ER), CAP, dtype=np.int32)
    disp_T = np.zeros((ER, H, CAP), dtype=np.float32)
    for e in range(ER):
        idx_e = np.nonzero(expert_mask[:, e])[0][:CAP]
        w_e = np.where(topi[idx_e] == e, topv[idx_e], 0).sum(1)
        imp = w_e * xnorm[idx_e]
        perm = np.argsort(-imp, kind="stable")       # important slots first
        disp_T[e, :, :len(idx_e)] = x_flat[idx_e[perm]].T
        pos[idx_e[perm], e] = np.arange(len(idx_e), dtype=np.int32)
    return pos, disp_T


def _pack(a, dtype, nrow):
    """[ER, nrow*P, width] -> contiguous [P, ER*nrow, width] in dtype."""
    w = a.shape[-1]
    return np.ascontiguousarray(
        a.reshape(ER, nrow, P, w).transpose(2, 0, 1, 3)
        .reshape(P, ER * nrow, w).astype(dtype))


def _make_in_maps(disp_T, experts_inter, experts_out, mm_dtype=None):
    """Per-core device input maps (I-sharded hi/lo fp8 weights,
    replicated hi/lo fp8 x)."""
    import ml_dtypes

    f8 = ml_dtypes.float8_e4m3
    f32 = np.float32
    maps0 = {}
    if NB:
        xa = disp_T[:, :, :NB]
        xh8 = xa.astype(f8)
        maps0["xh"] = _pack(xh8, f8, KO)
        maps0["xl"] = _pack(xa - xh8.astype(f32), f8, KO)
    if NQ:
        maps0["xb"] = _pack(disp_T[:, :, NB:], f8, KO)

    in_maps = []
    for c in range(N_CORES):
        sl = slice(c * ISL, (c + 1) * ISL)
        wi16 = experts_inter[:, :, sl] * WI_SCALE
        wo64 = experts_out[:, sl, :] * WO_SCALE
        wh8 = wi16.astype(f8)
        woh8 = wo64.astype(f8)
        m = dict(maps0)
        m["wh"] = _pack(wh8, f8, KO)
        m["woh"] = _pack(woh8, f8, ICH)
        if NB:
            m["wl"] = _pack(wi16 - wh8.astype(f32), f8, KO)
            m["wol"] = _pack(wo64 - woh8.astype(f32), f8, ICH)
        in_maps.append(m)
    return in_maps, 1.0


def kernel(x, router_w, router_b, experts_inter, experts_out):
    from concourse.bass_utils import run_bass_kernel_spmd

    x = np.ascontiguousarray(np.asarray(x, dtype=np.float32))
    router_w = np.asarray(router_w, dtype=np.float32)
    router_b = np.asarray(router_b, dtype=np.float32)
    experts_inter = np.asarray(experts_inter, dtype=np.float32)
    experts_out = np.asarray(experts_out, dtype=np.float32)

    x_flat = x.reshape(N_TOK, H)
    topv, topi = _routing(x_flat, router_w, router_b)
    pos, disp_T = _dispatch(x_flat, topv, topi)
    rows = np.arange(N_TOK)

    in_maps, _ = _make_in_maps(disp_T, experts_inter, experts_out)

    nc = _get_nc(MM_DTYPE)
    trace = bool(int(os.environ.get("MOE_TRACE", "0")))
    res = run_bass_kernel_spmd(nc, in_maps, list(range(N_CORES)), trace=trace)
    global LAST_RESULT
    LAST_RESULT = res

    # sum the 8 partial outputs -> [7, H, CAP]; undo the 16*64 fp8 scale
    acc = np.zeros((P, ER * KO, CAP), dtype=np.float32)
    for c in range(N_CORES):
        if NB:
            acc[:, :, :NB] += res.results[c]["ypa"]
        if NQ:
            acc[:, :, NB:] += res.results[c]["ypb"]
    acc *= OUT_SCALE
    out_T = np.ascontiguousarray(
        acc.reshape(P, ER, KO, CAP).transpose(1, 2, 0, 3).reshape(ER, H, CAP))

    # ---- host combine: pure gathers ----
    out_flat = np.ascontiguousarray(out_T.transpose(0, 2, 1)).reshape(
        ER * CAP, H)
    out_ext = np.vstack([out_flat, np.zeros((1, H), dtype=np.float32)])

    combined = np.zeros_like(x_flat)
    noop_w = np.zeros(N_TOK, dtype=np.float32)
    for k in range(TOP_K):
        e_k = topi[:, k]
        v_k = topv[:, k]
        is_noop = e_k == ER
        noop_w += np.where(is_noop, v_k, 0.0).astype(np.float32)
        p_k = pos[rows, np.minimum(e_k, ER - 1)]
        ok = (~is_noop) & (p_k < CAP)
        slot = np.where(ok, np.minimum(e_k, ER - 1) * CAP + p_k, ER * CAP)
        combined += out_ext[slot] * np.where(ok, v_k, 0.0)[:, None]
    combined += x_flat * noop_w[:, None]

    return combined.reshape(B, S, H)


# revision 18
# speedup vs baseline: 1.2587x; 1.2587x over previous
"""MoE routing kernel for Trainium2 (8 NeuronCores, I-sharded, mixed prec).

Problem: nn_MoDE_52140902973544 (moe_routing).
  x[4,2048,1024], router (8 experts, top-2, capacity 1024), 7 real experts
  with FFN H=1024 -> I=4096 -> H=1024 (relu), expert 7 = identity (noop).

Strategy:
  * Host: router forward + top-2 + capacity-limited dispatch (pure index
    math, order-based -> float-robust).  Within each expert the CAP
    dispatched slots are PERMUTED by importance (gate weight x token
    norm, descending): the combine is a gather so any permutation is
    exact; it lets the device compute the low-importance tail in fp8.
  * Device (SPMD over 8 cores): core c owns I-slice [c*512,(c+1)*512)
    of the FFN intermediate dim for ALL 7 experts (perfect balance,
    458,752 bf16-equivalent PE cycles/core, no collectives):
        h_e  = relu(x_e @ Wi_e[:, sl])       # exact (relu elementwise)
        yp_e = h_e @ Wo_e[sl, :]             # partial over I, fp32 out
    Each expert's tokens split into segment A (top importance, bf16)
    and segment B (tail, fp8e4 + DoubleRow perf mode = 2x PE
    throughput).  B's quantization error lands only on slots whose
    combined contribution is small: measured end-to-end error stays
    well under the 2e-2 gate while PE work drops by NQ/2048.
  * Host: sum the 8 fp32 partials, un-scale the fp8 columns, combine
    via pure gathers + gate weights + noop path.

fp8 scales (powers of 2, exponent-only): wi8 = wi*16, wo8 = wo*64 lift
the small expert weights out of the fp8e4 subnormal range; host divides
segment-B outputs by 1024.
"""

import os
import sys

for _p in ("/opt/trn_rl_repo", "/opt/pypackages"):
    if _p not in sys.path:
        sys.path.append(_p)

import numpy as np

# ---- problem constants (hardcoded per contract) ----
B, S, H, I = 4, 2048, 1024, 4096
E = 8                 # experts incl. noop (last)
ER = E - 1            # real experts
TOP_K = 2
N_TOK = B * S         # 8192
CAP = 1024            # ceil(N_TOK / E * 1.0)
N_CORES = 8

P = 128               # partitions
KO = H // P           # 8   H chunks
ISL = I // N_CORES    # 512 I-slice per core
ICH = ISL // P        # 4   I chunks per core
NF = 512              # max matmul free dim per PSUM tile (1 bank fp32)

# tokens per expert computed in fp8 (low-importance tail); 0 = pure bf16
NQ = int(os.environ.get("MOE_NQ", "512"))
NB = CAP - NQ
WI_SCALE = 16.0
WO_SCALE = 64.0
OUT_SCALE_B = 1.0 / (WI_SCALE * WO_SCALE)

MM_DTYPE = os.environ.get("MOE_MM_DTYPE", "bf16")

_CACHE = {}


def _free_tiles(n):
    """Split a free dim of n columns into <=NF chunks: [(off, width)...]"""
    out, off = [], 0
    while off < n:
        w = min(NF, n - off)
        out.append((off, w))
        off += w
    return out


def _build_nc(mm_dtype: str = "bf16", repeat: int = 1,
              loop_repeat: int | None = None, staggered: bool = False):
    """Single-core Bass program (SPMD across 8 cores, I-sharded).

    DRAM inputs host-packed with the partition dim first so every DMA is
    a plain contiguous slice:
      xa  [128, 7*8, NB]  bf16    xb  [128, 7*8, NQ]  fp8e4
      wi  [128, 7*8, 512] bf16    wi8 [128, 7*8, 512] fp8e4 (x16)
      wo  [128, 7*4, 1024] bf16   wo8 [128, 7*4, 1024] fp8e4 (x64)
      yp  [128, 7*8, 1024] fp32 out (cols [NB:] carry the x1024 scale)
    """
    import concourse.bacc as bacc
    import concourse.mybir as mybir
    import concourse.tile as tile

    dt = mybir.dt
    F8 = dt.float8e4
    DR = mybir.MatmulPerfMode.DoubleRow

    nc = bacc.Bacc("TRN2")
    xa = nc.declare_dram_parameter("xa", [P, ER * KO, NB], dt.bfloat16,
                                   isOutput=False) if NB else None
    xb = nc.declare_dram_parameter("xb", [P, ER * KO, NQ], F8,
                                   isOutput=False) if NQ else None
    wi = nc.declare_dram_parameter("wi", [P, ER * KO, ISL], dt.bfloat16,
                                   isOutput=False) if NB else None
    wi8 = nc.declare_dram_parameter("wi8", [P, ER * KO, ISL], F8,
                                    isOutput=False) if NQ else None
    wo = nc.declare_dram_parameter("wo", [P, ER * ICH, H], dt.bfloat16,
                                   isOutput=False) if NB else None
    wo8 = nc.declare_dram_parameter("wo8", [P, ER * ICH, H], F8,
                                    isOutput=False) if NQ else None
    ypa = nc.declare_dram_parameter("ypa", [P, ER * KO, NB], dt.bfloat16,
                                    isOutput=True) if NB else None
    ypb = nc.declare_dram_parameter("ypb", [P, ER * KO, NQ], dt.bfloat16,
                                    isOutput=True) if NQ else None

    # (tag, dtype, x dram, wi dram, wo dram, out dram, n tokens, kstep)
    segs = []
    if NB:
        segs.append(("a", dt.bfloat16, xa, wi, wo, ypa, NB, 1, None))
    if NQ:
        segs.append(("b", F8, xb, wi8, wo8, ypb, NQ, 2, DR))

    with tile.TileContext(nc) as tc:
        from contextlib import ExitStack

        with ExitStack() as ctx:
            xpool = ctx.enter_context(tc.tile_pool(name="x", bufs=2))
            wipool = ctx.enter_context(tc.tile_pool(name="wi", bufs=2))
            wopool = ctx.enter_context(tc.tile_pool(name="wo", bufs=2))
            hpool = ctx.enter_context(tc.tile_pool(name="h", bufs=2))
            opool = ctx.enter_context(tc.tile_pool(name="o", bufs=3))
            # 8 one-bank [P, <=512] fp32 tiles: drains of chain g overlap
            # matmuls of later chains
            pspool = ctx.enter_context(
                tc.tile_pool(name="ps", bufs=8, space="PSUM"))

            def _emit_g1(e, tag, DT, xd, wid, wod, ypd, ntok, kstep, perf):
                """DMAs + GEMM1 for one (expert, segment); returns tiles."""
                fts = _free_tiles(ntok)
                xt = xpool.tile([P, KO, ntok], DT, tag=f"x{tag}",
                                name=f"x{tag}{e}")
                nc.sync.dma_start(xt[:], xd[:, e * KO:(e + 1) * KO, :])
                wit = wipool.tile([P, KO, ISL], DT, tag=f"wi{tag}",
                                  name=f"wi{tag}{e}")
                nc.sync.dma_start(wit[:], wid[:, e * KO:(e + 1) * KO, :])
                wot = wopool.tile([P, ICH, H], DT, tag=f"wo{tag}",
                                  name=f"wo{tag}{e}")
                nc.sync.dma_start(wot[:], wod[:, e * ICH:(e + 1) * ICH, :])

                ht = hpool.tile([P, ICH, ntok], DT, tag=f"h{tag}",
                                name=f"h{tag}{e}")

                # ---- GEMM1: h = relu(Wi_sl.T @ X.T), I-chunk pairs ----
                for ir in range(0, ICH, 2):
                    ps = [[pspool.tile([P, w], dt.float32, tag="ps",
                                       name=f"ps1{tag}_{e}_{ir + di}_{oi}")
                           for oi, (off, w) in enumerate(fts)]
                          for di in range(2)]
                    for k in range(0, KO, kstep):
                        for di in range(2):
                            for oi, (off, w) in enumerate(fts):
                                nc.tensor.matmul(
                                    ps[di][oi][:],
                                    wit[:, k:k + kstep,
                                        (ir + di) * P:(ir + di + 1) * P],
                                    xt[:, k:k + kstep, off:off + w],
                                    start=(k == 0),
                                    stop=(k == KO - kstep),
                                    perf_mode=perf,
                                )
                    for di in range(2):
                        for oi, (off, w) in enumerate(fts):
                            nc.vector.tensor_scalar_max(
                                ht[:, ir + di, off:off + w], ps[di][oi][:],
                                0.0)
                return ht, wot

            def _emit_g2(e, ht, wot, tag, DT, xd, wid, wod, ypd, ntok,
                         kstep, perf):
                fts = _free_tiles(ntok)
                # ---- GEMM2: yp = Wo_sl.T @ h, H-chunk pairs ----
                for hr in range(0, KO, 2):
                    qs = [[pspool.tile([P, w], dt.float32, tag="ps",
                                       name=f"ps2{tag}_{e}_{hr + m}_{oi}")
                           for oi, (off, w) in enumerate(fts)]
                          for m in range(2)]
                    for k in range(0, ICH, kstep):
                        for m in range(2):
                            for oi, (off, w) in enumerate(fts):
                                nc.tensor.matmul(
                                    qs[m][oi][:],
                                    wot[:, k:k + kstep,
                                        (hr + m) * P:(hr + m + 1) * P],
                                    ht[:, k:k + kstep, off:off + w],
                                    start=(k == 0),
                                    stop=(k == ICH - kstep),
                                    perf_mode=perf,
                                )
                    ot = opool.tile([P, 2, ntok], dt.bfloat16, tag=f"o{tag}",
                                    name=f"o{tag}{e}_{hr}")
                    for m in range(2):
                        for oi, (off, w) in enumerate(fts):
                            nc.vector.tensor_copy(
                                ot[:, m, off:off + w], qs[m][oi][:])
                    nc.sync.dma_start(
                        ypd[:, e * KO + hr:e * KO + hr + 2, :],
                        ot[:])

            def _emit_body():
                # per expert: G1A, G1B, G2A, G2B -- segment B's relu
                # drains complete under G2A's long chains, so G2B never
                # stalls on the DVE
                for e in range(ER):
                    tiles = [_emit_g1(e, *seg) for seg in segs]
                    for (ht, wot), seg in zip(tiles, segs):
                        _emit_g2(e, ht, wot, *seg)

            if loop_repeat is not None:
                # device-side repeat loop for the slope timing method
                with tc.For_i(0, loop_repeat, 1,
                              hint_engines=(mybir.EngineType.PE,
                                            mybir.EngineType.DVE),
                              staggered_reset=staggered):
                    _emit_body()
            else:
                for _rep in range(repeat):
                    _emit_body()
    nc.compile()
    return nc


def _get_nc(mm_dtype: str):
    if mm_dtype not in _CACHE:
        _CACHE[mm_dtype] = _build_nc(mm_dtype)
    return _CACHE[mm_dtype]


def _routing(x_flat: np.ndarray, router_w: np.ndarray, router_b: np.ndarray):
    """Replicate the reference router bit-for-bit where possible (jax CPU),
    returning top-2 values/indices [N_TOK, 2] (fp32/int)."""
    try:
        import jax
        import jax.numpy as jnp

        cpu = jax.devices("cpu")[0]
        with jax.default_device(cpu):
            xj = jnp.asarray(x_flat.reshape(B, S, H))
            logits = jnp.einsum("bsh,eh->bse", xj, jnp.asarray(router_w)) \
                + jnp.asarray(router_b)
            wflat = jax.nn.softmax(logits, axis=-1).reshape(N_TOK, E)
            topv, topi = jax.lax.top_k(wflat, TOP_K)
            return np.asarray(topv), np.asarray(topi)
    except Exception:
        # numpy fallback (float64 logits for a stable ordering)
        logits = x_flat.astype(np.float64) @ router_w.astype(np.float64).T \
            + router_b.astype(np.float64)
        m = logits.max(axis=1, keepdims=True)
        ex = np.exp(logits - m)
        wflat = (ex / ex.sum(axis=1, keepdims=True)).astype(np.float32)
        topi = np.argsort(-wflat, axis=1, kind="stable")[:, :TOP_K]
        topv = np.take_along_axis(wflat, topi, axis=1)
        return topv, topi


def _dispatch(x_flat, topv, topi):
    """Capacity-limited dispatch (exact reference order semantics), with
    slots permuted by importance inside each expert.

    Returns (pos, disp_T): pos[t, e] = slot column of token t for expert
    e (importance-permuted); disp_T[e] = x of the first CAP selectors in
    importance order, transposed to [H, CAP]."""
    mask = np.zeros((N_TOK, E), dtype=bool)
    rows = np.arange(N_TOK)
    mask[rows[:, None], topi] = True
    expert_mask = mask[:, :ER]                       # [N, 7]
    rank = np.cumsum(expert_mask, axis=0, dtype=np.int32) - 1
    xnorm = np.linalg.norm(x_flat, axis=1)

    pos = np.full((N_TOK, ER), CAP, dtype=np.int32)
    disp_T = np.zeros((ER, H, CAP), dtype=np.float32)
    for e in range(ER):
        idx_e = np.nonzero(expert_mask[:, e])[0][:CAP]
        w_e = np.where(topi[idx_e] == e, topv[idx_e], 0).sum(1)
        imp = w_e * xnorm[idx_e]
        perm = np.argsort(-imp, kind="stable")       # important slots first
        disp_T[e, :, :len(idx_e)] = x_flat[idx_e[perm]].T
        pos[idx_e[perm], e] = np.arange(len(idx_e), dtype=np.int32)
    return pos, disp_T


def _pack(a, dtype, nrow):
    """[ER, nrow*P, width] -> contiguous [P, ER*nrow, width] in dtype."""
    w = a.shape[-1]
    return np.ascontiguousarray(
        a.reshape(ER, nrow, P, w).transpose(2, 0, 1, 3)
        .reshape(P, ER * nrow, w).astype(dtype))


def _make_in_maps(disp_T, experts_inter, experts_out, mm_dtype=None):
    """Per-core device input maps (I-sharded weights, replicated x)."""
    import ml_dtypes

    bf = ml_dtypes.bfloat16
    f8 = ml_dtypes.float8_e4m3
    maps0 = {}
    if NB:
        maps0["xa"] = _pack(disp_T[:, :, :NB], bf, KO)
    if NQ:
        maps0["xb"] = _pack(disp_T[:, :, NB:], f8, KO)

    in_maps = []
    for c in range(N_CORES):
        sl = slice(c * ISL, (c + 1) * ISL)
        wic = np.ascontiguousarray(experts_inter[:, :, sl])
        woc = np.ascontiguousarray(experts_out[:, sl, :])
        m = dict(maps0)
        if NB:
            m["wi"] = _pack(wic, bf, KO)
            m["wo"] = _pack(woc, bf, ICH)
        if NQ:
            m["wi8"] = _pack(wic * WI_SCALE, f8, KO)
            m["wo8"] = _pack(woc * WO_SCALE, f8, ICH)
        in_maps.append(m)
    return in_maps, 1.0


def kernel(x, router_w, router_b, experts_inter, experts_out):
    from concourse.bass_utils import run_bass_kernel_spmd

    x = np.ascontiguousarray(np.asarray(x, dtype=np.float32))
    router_w = np.asarray(router_w, dtype=np.float32)
    router_b = np.asarray(router_b, dtype=np.float32)
    experts_inter = np.asarray(experts_inter, dtype=np.float32)
    experts_out = np.asarray(experts_out, dtype=np.float32)

    x_flat = x.reshape(N_TOK, H)
    topv, topi = _routing(x_flat, router_w, router_b)
    pos, disp_T = _dispatch(x_flat, topv, topi)
    rows = np.arange(N_TOK)

    in_maps, _ = _make_in_maps(disp_T, experts_inter, experts_out)

    nc = _get_nc(MM_DTYPE)
    trace = bool(int(os.environ.get("MOE_TRACE", "0")))
    res = run_bass_kernel_spmd(nc, in_maps, list(range(N_CORES)), trace=trace)
    global LAST_RESULT
    LAST_RESULT = res

    # sum the 8 partial outputs -> [7, H, CAP]; un-scale the fp8 columns
    acc = np.zeros((P, ER * KO, CAP), dtype=np.float32)
    for c in range(N_CORES):
        if NB:
            acc[:, :, :NB] += res.results[c]["ypa"]
        if NQ:
            acc[:, :, NB:] += res.results[c]["ypb"]
    if NQ:
        acc[:, :, NB:] *= OUT_SCALE_B
    out_T = np.ascontiguousarray(
        acc.reshape(P, ER, KO, CAP).transpose(1, 2, 0, 3).reshape(ER, H, CAP))

    # ---- host combine: pure gathers ----
    out_flat = np.ascontiguousarray(out_T.transpose(0, 2, 1)).reshape(
        ER * CAP, H)
    out_ext = np.vstack([out_flat, np.zeros((1, H), dtype=np.float32)])

    combined = np.zeros_like(x_flat)
    noop_w = np.zeros(N_TOK, dtype=np.float32)
    for k in range(TOP_K):
        e_k = topi[:, k]
        v_k = topv[:, k]
        is_noop = e_k == ER
        noop_w += np.where(is_noop, v_k, 0.0).astype(np.float32)
        p_k = pos[rows, np.minimum(e_k, ER - 1)]
        ok = (~is_noop) & (p_k < CAP)
        slot = np.where(ok, np.minimum(e_k, ER - 1) * CAP + p_k, ER * CAP)
        combined += out_ext[slot] * np.where(ok, v_k, 0.0)[:, None]
    combined += x_flat * noop_w[:, None]

    return combined.reshape(B, S, H)


# revision 23
# speedup vs baseline: 1.2879x; 1.0232x over previous
"""MoE routing kernel for Trainium2 (8 NeuronCores, I-sharded, mixed prec).

Problem: nn_MoDE_52140902973544 (moe_routing).
  x[4,2048,1024], router (8 experts, top-2, capacity 1024), 7 real experts
  with FFN H=1024 -> I=4096 -> H=1024 (relu), expert 7 = identity (noop).

Strategy:
  * Host: router forward + top-2 + capacity-limited dispatch (pure index
    math, order-based -> float-robust).  Within each expert the CAP
    dispatched slots are PERMUTED by importance (gate weight x token
    norm, descending): the combine is a gather so any permutation is
    exact; it lets the device compute the low-importance tail in fp8.
  * Device (SPMD over 8 cores): core c owns I-slice [c*512,(c+1)*512)
    of the FFN intermediate dim for ALL 7 experts (perfect balance, no
    collectives; v1 wasted core 7 duplicating core 0):
        h_e  = relu(x_e @ Wi_e[:, sl])       # exact (relu elementwise)
        yp_e = h_e @ Wo_e[sl, :]             # partial over I, bf16 out
    Each expert's tokens split into segment A (top importance, bf16)
    and segment B (tail, fp8e4 + DoubleRow perf mode).  B's quantization
    error lands only on slots whose combined-output share is ~15%:
    measured end-to-end 1.68e-2 (< 2e-2 gate) while B's PE cost halves.
  * Host: sum the 8 bf16 partials in fp32, un-scale the fp8 columns,
    combine via pure gathers + gate weights + noop path.

PE roofline notes (measured, see transcript): a DoubleRow matmul still
takes 512 cycles per 512 output columns -- the moving-operand SBUF port
is 2 bytes/cycle/partition, so fp8+DR contracts 2 k-chunks per
instruction for a hard 2x ceiling (the CoreSim cost model's 0.5
cycles/row would imply 4x; hardware disagrees).  3-term compensated-fp8
(xh@Wh + xl@Wh + xh@Wl) therefore costs 1.5x bf16 and loses.  Per-core
floor = 7 experts * (32,768 bf16 + 16,384 fp8-DR cycle-slots) = 344,064
~= 192us at the ~1.79GHz sustained-power clock; this kernel measures
196-210us depending on chip clock state (dispatch-to-dispatch DVFS
swings of ~10% are visible in the R=513 legs).

fp8 scales (powers of 2, exponent-only): wi8 = wi*16, wo8 = wo*64 lift
the small expert weights out of the fp8e4 subnormal range; host divides
segment-B outputs by 1024.
"""

import os
import sys

for _p in ("/opt/trn_rl_repo", "/opt/pypackages"):
    if _p not in sys.path:
        sys.path.append(_p)

import numpy as np

# ---- problem constants (hardcoded per contract) ----
B, S, H, I = 4, 2048, 1024, 4096
E = 8                 # experts incl. noop (last)
ER = E - 1            # real experts
TOP_K = 2
N_TOK = B * S         # 8192
CAP = 1024            # ceil(N_TOK / E * 1.0)
N_CORES = 8

P = 128               # partitions
KO = H // P           # 8   H chunks
ISL = I // N_CORES    # 512 I-slice per core
ICH = ISL // P        # 4   I chunks per core
NF = 512              # max matmul free dim per PSUM tile (1 bank fp32)

# tokens per expert computed in fp8 (low-importance tail); 0 = pure bf16
NQ = int(os.environ.get("MOE_NQ", "512"))
NB = CAP - NQ
WI_SCALE = 16.0
WO_SCALE = 64.0
OUT_SCALE_B = 1.0 / (WI_SCALE * WO_SCALE)

MM_DTYPE = os.environ.get("MOE_MM_DTYPE", "bf16")

_CACHE = {}


def _free_tiles(n):
    """Split a free dim of n columns into <=NF chunks: [(off, width)...]"""
    out, off = [], 0
    while off < n:
        w = min(NF, n - off)
        out.append((off, w))
        off += w
    return out


def _build_nc(mm_dtype: str = "bf16", repeat: int = 1,
              loop_repeat: int | None = None, staggered: bool = False,
              ablate: str = "full"):
    """Single-core Bass program (SPMD across 8 cores, I-sharded).

    DRAM inputs host-packed with the partition dim first so every DMA is
    a plain contiguous slice:
      xa  [128, 7*8, NB]  bf16    xb  [128, 7*8, NQ]  fp8e4
      wi  [128, 7*8, 512] bf16    wi8 [128, 7*8, 512] fp8e4 (x16)
      wo  [128, 7*4, 1024] bf16   wo8 [128, 7*4, 1024] fp8e4 (x64)
      yp  [128, 7*8, 1024] fp32 out (cols [NB:] carry the x1024 scale)
    """
    import concourse.bacc as bacc
    import concourse.mybir as mybir
    import concourse.tile as tile

    dt = mybir.dt
    F8 = dt.float8e4
    DR = mybir.MatmulPerfMode.DoubleRow

    nc = bacc.Bacc("TRN2")
    xa = nc.declare_dram_parameter("xa", [P, ER * KO, NB], dt.bfloat16,
                                   isOutput=False) if NB else None
    xb = nc.declare_dram_parameter("xb", [P, ER * KO, NQ], F8,
                                   isOutput=False) if NQ else None
    wi = nc.declare_dram_parameter("wi", [P, ER * KO, ISL], dt.bfloat16,
                                   isOutput=False) if NB else None
    wi8 = nc.declare_dram_parameter("wi8", [P, ER * KO, ISL], F8,
                                    isOutput=False) if NQ else None
    wo = nc.declare_dram_parameter("wo", [P, ER * ICH, H], dt.bfloat16,
                                   isOutput=False) if NB else None
    wo8 = nc.declare_dram_parameter("wo8", [P, ER * ICH, H], F8,
                                    isOutput=False) if NQ else None
    ypa = nc.declare_dram_parameter("ypa", [P, ER * KO, NB], dt.bfloat16,
                                    isOutput=True) if NB else None
    ypb = nc.declare_dram_parameter("ypb", [P, ER * KO, NQ], dt.bfloat16,
                                    isOutput=True) if NQ else None

    # (tag, dtype, x dram, wi dram, wo dram, out dram, n tokens, kstep)
    segs = []
    if NB:
        segs.append(("a", dt.bfloat16, xa, wi, wo, ypa, NB, 1, None))
    if NQ:
        segs.append(("b", F8, xb, wi8, wo8, ypb, NQ, 2, DR))

    with tile.TileContext(nc) as tc:
        from contextlib import ExitStack

        with ExitStack() as ctx:
            xpool = ctx.enter_context(tc.tile_pool(name="x", bufs=2))
            wipool = ctx.enter_context(tc.tile_pool(name="wi", bufs=2))
            wopool = ctx.enter_context(tc.tile_pool(name="wo", bufs=2))
            hpool = ctx.enter_context(tc.tile_pool(name="h", bufs=2))
            opool = ctx.enter_context(tc.tile_pool(name="o", bufs=3))
            # 8 one-bank [P, <=512] fp32 tiles: drains of chain g overlap
            # matmuls of later chains
            pspool = ctx.enter_context(
                tc.tile_pool(name="ps", bufs=8, space="PSUM"))

            do_dma = ablate not in ("pe", "nodma")
            do_drain = ablate != "pe"

            def _emit_g1(e, tag, DT, xd, wid, wod, ypd, ntok, kstep, perf):
                """DMAs + GEMM1 for one (expert, segment); returns tiles."""
                fts = _free_tiles(ntok)
                xt = xpool.tile([P, KO, ntok], DT, tag=f"x{tag}",
                                name=f"x{tag}{e}")
                wit = wipool.tile([P, KO, ISL], DT, tag=f"wi{tag}",
                                  name=f"wi{tag}{e}")
                wot = wopool.tile([P, ICH, H], DT, tag=f"wo{tag}",
                                  name=f"wo{tag}{e}")
                if do_dma:
                    nc.sync.dma_start(xt[:], xd[:, e * KO:(e + 1) * KO, :])
                    nc.sync.dma_start(wit[:], wid[:, e * KO:(e + 1) * KO, :])
                    nc.sync.dma_start(wot[:], wod[:, e * ICH:(e + 1) * ICH, :])

                ht = hpool.tile([P, ICH, ntok], DT, tag=f"h{tag}",
                                name=f"h{tag}{e}")

                # ---- GEMM1: h = relu(Wi_sl.T @ X.T), I-chunk pairs ----
                for ir in range(0, ICH, 2):
                    ps = [[pspool.tile([P, w], dt.float32, tag="ps",
                                       name=f"ps1{tag}_{e}_{ir + di}_{oi}")
                           for oi, (off, w) in enumerate(fts)]
                          for di in range(2)]
                    for k in range(0, KO, kstep):
                        for di in range(2):
                            for oi, (off, w) in enumerate(fts):
                                nc.tensor.matmul(
                                    ps[di][oi][:],
                                    wit[:, k:k + kstep,
                                        (ir + di) * P:(ir + di + 1) * P],
                                    xt[:, k:k + kstep, off:off + w],
                                    start=(k == 0),
                                    stop=(k == KO - kstep),
                                    perf_mode=perf,
                                )
                    if do_drain:
                        for di in range(2):
                            for oi, (off, w) in enumerate(fts):
                                nc.vector.tensor_scalar_max(
                                    ht[:, ir + di, off:off + w],
                                    ps[di][oi][:], 0.0)
                return ht, wot

            def _emit_g2(e, ht, wot, tag, DT, xd, wid, wod, ypd, ntok,
                         kstep, perf):
                fts = _free_tiles(ntok)
                # ---- GEMM2: yp = Wo_sl.T @ h, H-chunk pairs ----
                for hr in range(0, KO, 2):
                    qs = [[pspool.tile([P, w], dt.float32, tag="ps",
                                       name=f"ps2{tag}_{e}_{hr + m}_{oi}")
                           for oi, (off, w) in enumerate(fts)]
                          for m in range(2)]
                    for k in range(0, ICH, kstep):
                        for m in range(2):
                            for oi, (off, w) in enumerate(fts):
                                nc.tensor.matmul(
                                    qs[m][oi][:],
                                    wot[:, k:k + kstep,
                                        (hr + m) * P:(hr + m + 1) * P],
                                    ht[:, k:k + kstep, off:off + w],
                                    start=(k == 0),
                                    stop=(k == ICH - kstep),
                                    perf_mode=perf,
                                )
                    if not do_drain:
                        continue
                    ot = opool.tile([P, 2, ntok], dt.bfloat16, tag=f"o{tag}",
                                    name=f"o{tag}{e}_{hr}")
                    for m in range(2):
                        for oi, (off, w) in enumerate(fts):
                            nc.vector.tensor_copy(
                                ot[:, m, off:off + w], qs[m][oi][:])
                    if do_dma:
                        nc.sync.dma_start(
                            ypd[:, e * KO + hr:e * KO + hr + 2, :],
                            ot[:])

            def _emit_body():
                # per expert: G1A, G1B, G2A, G2B -- segment B's relu
                # drains complete under G2A's long chains, so G2B never
                # stalls on the DVE
                for e in range(ER):
                    tiles = [_emit_g1(e, *seg) for seg in segs]
                    for (ht, wot), seg in zip(tiles, segs):
                        _emit_g2(e, ht, wot, *seg)

            if loop_repeat is not None:
                # device-side repeat loop for the slope timing method
                with tc.For_i(0, loop_repeat, 1,
                              hint_engines=(mybir.EngineType.PE,
                                            mybir.EngineType.DVE),
                              staggered_reset=staggered):
                    _emit_body()
            else:
                for _rep in range(repeat):
                    _emit_body()
    nc.compile()
    return nc


def _get_nc(mm_dtype: str):
    if mm_dtype not in _CACHE:
        _CACHE[mm_dtype] = _build_nc(mm_dtype)
    return _CACHE[mm_dtype]


def _routing(x_flat: np.ndarray, router_w: np.ndarray, router_b: np.ndarray):
    """Replicate the reference router bit-for-bit where possible (jax CPU),
    returning top-2 values/indices [N_TOK, 2] (fp32/int)."""
    try:
        import jax
        import jax.numpy as jnp

        cpu = jax.devices("cpu")[0]
        with jax.default_device(cpu):
            xj = jnp.asarray(x_flat.reshape(B, S, H))
            logits = jnp.einsum("bsh,eh->bse", xj, jnp.asarray(router_w)) \
                + jnp.asarray(router_b)
            wflat = jax.nn.softmax(logits, axis=-1).reshape(N_TOK, E)
            topv, topi = jax.lax.top_k(wflat, TOP_K)
            return np.asarray(topv), np.asarray(topi)
    except Exception:
        # numpy fallback (float64 logits for a stable ordering)
        logits = x_flat.astype(np.float64) @ router_w.astype(np.float64).T \
            + router_b.astype(np.float64)
        m = logits.max(axis=1, keepdims=True)
        ex = np.exp(logits - m)
        wflat = (ex / ex.sum(axis=1, keepdims=True)).astype(np.float32)
        topi = np.argsort(-wflat, axis=1, kind="stable")[:, :TOP_K]
        topv = np.take_along_axis(wflat, topi, axis=1)
        return topv, topi


def _dispatch(x_flat, topv, topi):
    """Capacity-limited dispatch (exact reference order semantics), with
    slots permuted by importance inside each expert.

    Returns (pos, disp_T): pos[t, e] = slot column of token t for expert
    e (importance-permuted); disp_T[e] = x of the first CAP selectors in
    importance order, transposed to [H, CAP]."""
    mask = np.zeros((N_TOK, E), dtype=bool)
    rows = np.arange(N_TOK)
    mask[rows[:, None], topi] = True
    expert_mask = mask[:, :ER]                       # [N, 7]
    rank = np.cumsum(expert_mask, axis=0, dtype=np.int32) - 1
    xnorm = np.linalg.norm(x_flat, axis=1)

    pos = np.full((N_TOK, ER), CAP, dtype=np.int32)
    disp_T = np.zeros((ER, H, CAP), dtype=np.float32)
    for e in range(ER):
        idx_e = np.nonzero(expert_mask[:, e])[0][:CAP]
        w_e = np.where(topi[idx_e] == e, topv[idx_e], 0).sum(1)
        imp = w_e * xnorm[idx_e]
        perm = np.argsort(-imp, kind="stable")       # important slots first
        disp_T[e, :, :len(idx_e)] = x_flat[idx_e[perm]].T
        pos[idx_e[perm], e] = np.arange(len(idx_e), dtype=np.int32)
    return pos, disp_T


def _pack(a, dtype, nrow):
    """[ER, nrow*P, width] -> contiguous [P, ER*nrow, width] in dtype."""
    w = a.shape[-1]
    return np.ascontiguousarray(
        a.reshape(ER, nrow, P, w).transpose(2, 0, 1, 3)
        .reshape(P, ER * nrow, w).astype(dtype))


def _make_in_maps(disp_T, experts_inter, experts_out, mm_dtype=None):
    """Per-core device input maps (I-sharded weights, replicated x)."""
    import ml_dtypes

    bf = ml_dtypes.bfloat16
    f8 = ml_dtypes.float8_e4m3
    maps0 = {}
    if NB:
        maps0["xa"] = _pack(disp_T[:, :, :NB], bf, KO)
    if NQ:
        maps0["xb"] = _pack(disp_T[:, :, NB:], f8, KO)

    in_maps = []
    for c in range(N_CORES):
        sl = slice(c * ISL, (c + 1) * ISL)
        wic = np.ascontiguousarray(experts_inter[:, :, sl])
        woc = np.ascontiguousarray(experts_out[:, sl, :])
        m = dict(maps0)
        if NB:
            m["wi"] = _pack(wic, bf, KO)
            m["wo"] = _pack(woc, bf, ICH)
        if NQ:
            m["wi8"] = _pack(wic * WI_SCALE, f8, KO)
            m["wo8"] = _pack(woc * WO_SCALE, f8, ICH)
        in_maps.append(m)
    return in_maps, 1.0


def kernel(x, router_w, router_b, experts_inter, experts_out):
    from concourse.bass_utils import run_bass_kernel_spmd

    x = np.ascontiguousarray(np.asarray(x, dtype=np.float32))
    router_w = np.asarray(router_w, dtype=np.float32)
    router_b = np.asarray(router_b, dtype=np.float32)
    experts_inter = np.asarray(experts_inter, dtype=np.float32)
    experts_out = np.asarray(experts_out, dtype=np.float32)

    x_flat = x.reshape(N_TOK, H)
    topv, topi = _routing(x_flat, router_w, router_b)
    pos, disp_T = _dispatch(x_flat, topv, topi)
    rows = np.arange(N_TOK)

    in_maps, _ = _make_in_maps(disp_T, experts_inter, experts_out)

    nc = _get_nc(MM_DTYPE)
    trace = bool(int(os.environ.get("MOE_TRACE", "0")))
    res = run_bass_kernel_spmd(nc, in_maps, list(range(N_CORES)), trace=trace)
    global LAST_RESULT
    LAST_RESULT = res

    # sum the 8 partial outputs -> [7, H, CAP]; un-scale the fp8 columns
    acc = np.zeros((P, ER * KO, CAP), dtype=np.float32)
    for c in range(N_CORES):
        if NB:
            acc[:, :, :NB] += res.results[c]["ypa"]
        if NQ:
            acc[:, :, NB:] += res.results[c]["ypb"]
    if NQ:
        acc[:, :, NB:] *= OUT_SCALE_B
    out_T = np.ascontiguousarray(
        acc.reshape(P, ER, KO, CAP).transpose(1, 2, 0, 3).reshape(ER, H, CAP))

    # ---- host combine: pure gathers ----
    out_flat = np.ascontiguousarray(out_T.transpose(0, 2, 1)).reshape(
        ER * CAP, H)
    out_ext = np.vstack([out_flat, np.zeros((1, H), dtype=np.float32)])

    combined = np.zeros_like(x_flat)
    noop_w = np.zeros(N_TOK, dtype=np.float32)
    for k in range(TOP_K):
        e_k = topi[:, k]
        v_k = topv[:, k]
        is_noop = e_k == ER
        noop_w += np.where(is_noop, v_k, 0.0).astype(np.float32)
        p_k = pos[rows, np.minimum(e_k, ER - 1)]
        ok = (~is_noop) & (p_k < CAP)
        slot = np.where(ok, np.minimum(e_k, ER - 1) * CAP + p_k, ER * CAP)
        combined += out_ext[slot] * np.where(ok, v_k, 0.0)[:, None]
    combined += x_flat * noop_w[:, None]

    return combined.reshape(B, S, H)
